# revision 34
# baseline (speedup 1.0000x reference)
"""CIN (Compressed Interaction Network) kernel for Trainium2, 8 NeuronCores.

Computes, per reference:
    x0 = xi = x                                  # (8192, 64, 32) fp32
    for (K, b) in layers:                        # K: (k, g, h)
        fm = relu(einsum('bgd,bhd,kgh->bkd', xi, x0, K) + b)
        pooled_i = fm.sum(-1); xi = fm
    out = concat(pooled, -1)                     # (8192, 384) fp32

Strategy (data-parallel over batch across 8 cores, 1024 rows each):
  Per n-tile (N=512 = 16 batch x 32 depth positions), the bilinear term is a
  single long PE accumulation  fm = Kperm @ Z  over 128-row "chunks" of
  Z[(g,h), n] = xi[g,n] * x0[h,n]  (bf16).  Z chunks are formed mostly on the
  Vector engine as wide tensor_tensor multiplies between the running feature
  map and "B tiles" holding broadcast/rotated copies of x0 rows, produced for
  free by the DMA engines straight from DRAM (0-stride / sliding-window access
  patterns).  A slice of layer-0's Z chunks (whose inputs are pure DMA
  products, available at tile start) is offloaded to the idle GPSIMD/Pool
  engine, interleaved v8/p4 so the PE never waits on the slower producer.
  Layer 0 exploits Z = x0 (x) x0 symmetry: K is folded on the host into 33
  diagonal bands, nearly halving layer-0 work.  ReLU+bias runs on the Scalar
  engine out of PSUM; per-layer emission is split into a main phase (Z +
  matmuls + ReLU + half-swap DMA) and a deferred post phase (accum_out pooled
  sums + output DMA) so the next tile's ReLU — on the fm critical path — is
  never queued behind off-path pooling work on the Scalar engine.  Weights
  stay resident in SBUF (bf16, host-permuted).
"""
import numpy as np
import ml_dtypes

import concourse.mybir as mybir
from concourse import bacc, tile

BF16 = mybir.dt.bfloat16
F32 = mybir.dt.float32
N = 512           # n-tile: 16 b * 32 d
NB = 16           # batch rows per tile
BCORE = 1024      # batch rows per core
NTILES = BCORE // NB
NCORES = 8

_CFG = dict(z_bufs=6, b_bufs=2, f_bufs=2, p_bufs=4, zway=8, b_split=1,
            waves=2, swap_dma=1, l0sym=1, act_pool=2, pool_w=4,
            zp_bufs=3, pool_l0=8, pool_tail=0)


def _v3(ap):
    return ap.rearrange("p (b d) -> p b d", d=32)


def _band_src(xt, js, j0, j1, b0):
    """Sliding-window src AP: out[p, j-j0, b, d] = xt[js + 2*j + p, b0+b, d]."""
    nj = j1 - j0
    base = js + 2 * j0
    s = xt[base:base+64, b0:b0+16, :].unsqueeze(1)
    row = BCORE * 32
    s.ap = mybir.VecI64Pair([[row, 64], [2 * row, nj], [32, 16], [1, 32]])
    return s


def _plan(n, pool_cnt, vway, pway):
    """Segment n chunks into interleaved (engine, width) runs: a 'v' run of
    vway, then a 'p' run (≤ pway) while pool quota remains, repeating.
    First run is always DVE so a layer never opens on the slow engine."""
    segs = []
    c, p_left = 0, pool_cnt
    while c < n:
        w = min(vway, n - c - p_left)
        if w > 0:
            segs.append(("v", w))
            c += w
        if p_left and c < n:
            w = min(pway, p_left, n - c)
            segs.append(("p", w))
            c += w
            p_left -= w
    return segs


def _plan_late(n, pool_cnt, vway, pway):
    """Pool runs placed LATE: DVE covers the head, Pool covers pool_cnt
    chunks just before a final DVE run of vway. Pool chunks are consumed
    mid-layer (max head-start after the fm dependency clears) and the
    stop-accumulation chunk stays DVE-produced."""
    if not pool_cnt:
        return _plan(n, 0, vway, pway)
    head = n - pool_cnt - vway
    assert head >= 0
    segs = []
    c = 0
    while c < head:
        w = min(vway, head - c)
        segs.append(("v", w))
        c += w
    p_left = pool_cnt
    while p_left:
        w = min(pway, p_left)
        segs.append(("p", w))
        p_left -= w
    segs.append(("v", vway))
    return segs


def _build(reps=1, ntiles=NTILES, z_bufs=4, b_bufs=2, zway=4, b_split=2,
           waves=1, f_bufs=2, p_bufs=3, swap_dma=0, l0sym=0, act_pool=0,
           pool_w=0, zp_bufs=3, pool_l0=0, pool_tail=0):
    nc = bacc.Bacc("TRN2", target_bir_lowering=False, debug=False)
    nxt = 64 + 33 if l0sym else 64
    xt = nc.dram_tensor("xt", (nxt, BCORE, 32), BF16, kind="ExternalInput").ap()
    nkf0 = 17 if l0sym else 32
    kf0 = nc.dram_tensor("kf0", (128, nkf0, 128), BF16, kind="ExternalInput").ap()
    kf1 = nc.dram_tensor("kf1", (128, 64, 128), BF16, kind="ExternalInput").ap()
    kf2 = nc.dram_tensor("kf2", (128, 64, 128), BF16, kind="ExternalInput").ap()
    bias = nc.dram_tensor("bias", (128, 3), F32, kind="ExternalInput").ap()
    pout = [nc.dram_tensor(f"p{i}", (128, BCORE), F32, kind="ExternalOutput").ap()
            for i in range(3)]

    with tile.TileContext(nc) as tc:
        from contextlib import ExitStack
        with ExitStack() as ctx:
            kpool = ctx.enter_context(tc.tile_pool(name="konst", bufs=1))
            bpool = ctx.enter_context(tc.tile_pool(name="bb", bufs=b_bufs))
            xpool = ctx.enter_context(tc.tile_pool(name="xx", bufs=waves + 1))
            fpool = ctx.enter_context(tc.tile_pool(name="fm", bufs=f_bufs))
            zpool = ctx.enter_context(tc.tile_pool(name="zz", bufs=z_bufs))
            zppool = (ctx.enter_context(tc.tile_pool(name="zzp", bufs=zp_bufs))
                      if pool_w else None)
            opool = ctx.enter_context(tc.tile_pool(name="oo", bufs=3))
            ppool = ctx.enter_context(tc.tile_pool(name="ps", bufs=p_bufs,
                                                   space="PSUM"))

            kt = [kpool.tile([128, nkf0, 128], BF16, tag="kf0", name="kt0"),
                  kpool.tile([128, 64, 128], BF16, tag="kf1", name="kt1"),
                  kpool.tile([128, 64, 128], BF16, tag="kf2", name="kt2")]
            nc.scalar.dma_start(kt[0][:], kf0[:])
            nc.scalar.dma_start(kt[1][:], kf1[:])
            nc.scalar.dma_start(kt[2][:], kf2[:])
            bt = kpool.tile([128, 3], F32, tag="bias", name="bt")
            nc.scalar.dma_start(bt[:], bias[:])

            def tile_dma(t):
                b0 = NB * t
                x2 = xpool.tile([128, N], BF16, tag="x2", name="x2")
                nc.sync.dma_start(_v3(x2[0:64, :]), xt[0:64, b0:b0+NB, :])
                nc.sync.dma_start(_v3(x2[64:128, :]), xt[0:64, b0:b0+NB, :])
                B0 = None
                if l0sym:
                    # B0[p<64, j] = x0[(p+2j)%64]; B0[p>=64, j] = x0[(p-64+2j+1)%64]
                    B0 = bpool.tile([128, 17, N], BF16, tag="B0", name="B0")
                    for lo, hi, js in ((0, 64, 0), (64, 128, 1)):
                        for j0, j1 in ((0, 9), (9, 17)):
                            nc.sync.dma_start(
                                B0[lo:hi, j0:j1, :].rearrange(
                                    "p j (b d) -> p j b d", d=32),
                                _band_src(xt, js, j0, j1, b0))
                # B[p<64, c] = x0[2c] bcast; B[p>=64, c] = x0[2c+1] bcast
                B = bpool.tile([128, 32, N], BF16, tag="B", name="B")
                cs = 32 // b_split
                for s in range(b_split):
                    c0 = cs * s
                    src_e = xt[2*c0:2*(c0+cs):2, b0:b0+NB, :].unsqueeze(0)
                    src_o = xt[2*c0+1:2*(c0+cs):2, b0:b0+NB, :].unsqueeze(0)
                    nc.sync.dma_start(
                        B[0:64, c0:c0+cs, :].rearrange("p c (b d) -> p c b d", d=32),
                        src_e.to_broadcast((64, cs, NB, 32)))
                    nc.sync.dma_start(
                        B[64:128, c0:c0+cs, :].rearrange("p c (b d) -> p c b d", d=32),
                        src_o.to_broadcast((64, cs, NB, 32)))
                return {"x2": x2, "B": B, "B0": B0, "fms": [], "psums": {}}

            def z_chunks(B, a_tile, kf_tile, kf_off, psum, start, stop, nchunk,
                         plan):
                c = 0
                for eng, w in plan:
                    if eng == "v":
                        z = zpool.tile([128, zway, N], BF16, tag="z", name="z")
                        nc.vector.tensor_tensor(
                            z[:, 0:w, :],
                            a_tile[:].unsqueeze(1).to_broadcast((128, w, N)),
                            B[:, c:c+w, :], mybir.AluOpType.mult)
                    else:
                        z = zppool.tile([128, pool_w, N], BF16, tag="zp",
                                        name="zp")
                        nc.gpsimd.tensor_tensor(
                            z[:, 0:w, :],
                            a_tile[:].unsqueeze(1).to_broadcast((128, w, N)),
                            B[:, c:c+w, :], mybir.AluOpType.mult)
                    for i in range(w):
                        nc.tensor.matmul(
                            psum[:], kf_tile[:, kf_off + c + i, :], z[:, i, :],
                            start=(start and c + i == 0),
                            stop=(stop and c + i == nchunk - 1))
                    c += w
                assert c == nchunk

            def tile_layer_main(t, layer, st):
                psum = ppool.tile([128, N], F32, tag="psum", name="psum")
                if layer == 0:
                    if l0sym:
                        z_chunks(st["B0"], st["x2"], kt[0], 0, psum, True, True,
                                 17, _plan(17, pool_l0, zway, pool_w))
                    else:
                        z_chunks(st["B"], st["x2"], kt[0], 0, psum, True, True,
                                 32, _plan(32, pool_l0, zway, pool_w))
                else:
                    fm_prev, fm_prev_s = st["fms"][-1]
                    z_chunks(st["B"], fm_prev, kt[layer], 0, psum, True, False,
                             32, _plan_late(32, pool_tail, zway, pool_w))
                    z_chunks(st["B"], fm_prev_s, kt[layer], 32, psum, False,
                             True, 32, _plan_late(32, pool_tail, zway, pool_w))
                fm = fpool.tile([128, N], BF16, tag=f"fm{layer}", name="fm")
                nc.scalar.activation(fm[:], psum[:],
                                     mybir.ActivationFunctionType.Relu,
                                     bias=bt[:, layer:layer+1], scale=1.0)
                if layer < 2:
                    fm_s = fpool.tile([128, N], BF16, tag=f"fms{layer}", name="fms")
                    if swap_dma:
                        nc.scalar.dma_start(fm_s[0:64, :], fm[64:128, :])
                        nc.scalar.dma_start(fm_s[64:128, :], fm[0:64, :])
                    else:
                        nc.vector.tensor_copy(fm_s[0:64, :], fm[64:128, :])
                        nc.vector.tensor_copy(fm_s[64:128, :], fm[0:64, :])
                    st["fms"].append((fm, fm_s))
                st["psums"][layer] = (psum, fm)

            def tile_layer_post(t, layer, st):
                b0 = NB * t
                psum, fm = st["psums"][layer]
                po = opool.tile([128, NB], F32, tag="po", name="po")
                if act_pool == 2:
                    scr = opool.tile([128, 32], BF16, tag="scr", name="scr")
                    for bb in range(NB):
                        nc.scalar.activation(scr[:],
                                             psum[:, 32*bb:32*(bb+1)],
                                             mybir.ActivationFunctionType.Relu,
                                             bias=bt[:, layer:layer+1],
                                             scale=1.0,
                                             accum_out=po[:, bb:bb+1])
                else:
                    nc.vector.tensor_reduce(po[:], _v3(fm[:]),
                                            axis=mybir.AxisListType.X,
                                            op=mybir.AluOpType.add)
                nc.sync.dma_start(pout[layer][:, b0:b0+NB], po[:])

            def emit_all():
                for base in range(0, ntiles, waves):
                    ts = [base + w for w in range(waves) if base + w < ntiles]
                    states = [tile_dma(t) for t in ts]
                    for layer in range(3):
                        for t, st in zip(ts, states):
                            tile_layer_main(t, layer, st)
                        for t, st in zip(ts, states):
                            tile_layer_post(t, layer, st)

            if reps > 1:
                with tc.For_i(0, reps, 1):
                    emit_all()
            else:
                emit_all()

    nc.compile()
    return nc


def _prep_inputs(x_shard, k0, k1, k2, b0, b1, b2, l0sym):
    xt = np.ascontiguousarray(x_shard.transpose(1, 0, 2)).astype(ml_dtypes.bfloat16)
    if l0sym:
        xt = np.concatenate([xt, xt[0:33]], axis=0)

    def perm0(K):
        KT = K.astype(np.float32)
        out = np.empty((128, 32, 128), np.float32)
        for c in range(32):
            out[0:64, c, :] = KT[:, :, 2*c].T
            out[64:128, c, :] = KT[:, :, 2*c+1].T
        return out.astype(ml_dtypes.bfloat16)

    def perm0_sym(K):
        # 33 diagonal bands (d=0..32) + zero pad band; chunk j = bands (2j, 2j+1)
        Kf = K.astype(np.float32)
        g = np.arange(64)
        bands = np.zeros((34, 128, 64), np.float32)
        for d in range(33):
            h = (g + d) % 64
            if d == 0:
                bands[d] = Kf[:, g, g]
            elif d == 32:
                bands[d] = Kf[:, g, h]
            else:
                bands[d] = Kf[:, g, h] + Kf[:, h, g]
        out = np.zeros((128, 17, 128), np.float32)
        for j in range(17):
            out[0:64, j, :] = bands[2*j].T
            if 2*j + 1 < 34:
                out[64:128, j, :] = bands[2*j+1].T
        return out.astype(ml_dtypes.bfloat16)

    def perm12(K):
        # normal chunk c: [(g 0:64, h=2c); (g 64:128, h=2c+1)]
        # swap   chunk c: [(g 64:128, h=2c); (g 0:64, h=2c+1)]  (A = fm halves swapped)
        KT = K.astype(np.float32)
        out = np.empty((128, 64, 128), np.float32)
        for c in range(32):
            e = KT[:, :, 2*c].T
            o = KT[:, :, 2*c+1].T
            out[0:64, c, :] = e[0:64]
            out[64:128, c, :] = o[64:128]
            out[0:64, 32+c, :] = e[64:128]
            out[64:128, 32+c, :] = o[0:64]
        return out.astype(ml_dtypes.bfloat16)

    bias = np.stack([np.broadcast_to(b, (128,)) for b in (b0, b1, b2)],
                    axis=1).astype(np.float32)
    return {"xt": xt, "kf0": (perm0_sym(k0) if l0sym else perm0(k0)),
            "kf1": perm12(k1), "kf2": perm12(k2),
            "bias": np.ascontiguousarray(bias)}


_cache = {}


def _get_runner():
    """Build + compile the Bass module and a reusable jitted SPMD runner."""
    if "runner" in _cache:
        return _cache["runner"]
    import jax
    from jax.sharding import Mesh, PartitionSpec
    from jax.experimental.shard_map import shard_map
    from concourse import bass2jax
    from concourse.bass2jax import _bass_exec_p, partition_id_tensor

    nc = _build(reps=1, ntiles=NTILES, **_CFG)
    bass2jax.install_neuronx_cc_hook()

    partition_name = nc.partition_id_tensor.name if nc.partition_id_tensor else None
    in_names, out_names, out_avals, zero_outs = [], [], [], []
    for alloc in nc.m.functions[0].allocations:
        if not isinstance(alloc, mybir.MemoryLocationSet):
            continue
        name = alloc.memorylocations[0].name
        if alloc.kind == "ExternalInput":
            if name != partition_name:
                in_names.append(name)
        elif alloc.kind == "ExternalOutput":
            out_names.append(name)
            shape = tuple(alloc.tensor_shape)
            dtype = mybir.dt.np(alloc.dtype)
            out_avals.append(jax.core.ShapedArray(shape, dtype))
            zero_outs.append(np.zeros(shape, dtype))
    n_params = len(in_names)
    n_outs = len(out_avals)
    in_names_all = in_names + out_names
    if partition_name is not None:
        in_names_all = in_names_all + [partition_name]
    donate = tuple(range(n_params, n_params + n_outs))

    def _body(*args):
        operands = list(args)
        if partition_name is not None:
            operands.append(partition_id_tensor())
        outs = _bass_exec_p.bind(
            *operands,
            out_avals=tuple(out_avals),
            in_names=tuple(in_names_all),
            out_names=tuple(out_names),
            lowering_input_output_aliases=(),
            sim_require_finite=True,
            sim_require_nnan=True,
            nc=nc,
        )
        return tuple(outs)

    devices = jax.devices()[:NCORES]
    assert len(devices) == NCORES, f"need {NCORES} devices, have {len(devices)}"
    mesh = Mesh(np.asarray(devices), ("core",))
    in_specs = (PartitionSpec("core"),) * (n_params + n_outs)
    out_specs = (PartitionSpec("core"),) * len(out_names)
    sharded = jax.jit(
        shard_map(_body, mesh=mesh, in_specs=in_specs, out_specs=out_specs,
                  check_rep=False),
        donate_argnums=donate, keep_unused=True)

    def run(in_maps):
        per_core = [[np.asarray(m[name]) for name in in_names] for m in in_maps]
        concat_in = [
            np.concatenate([per_core[c][i] for c in range(NCORES)], axis=0)
            for i in range(n_params)
        ]
        concat_zeros = [
            np.zeros((NCORES * z.shape[0], *z.shape[1:]), z.dtype)
            for z in zero_outs
        ]
        out_arrs = sharded(*concat_in, *concat_zeros)
        jax.block_until_ready(out_arrs)
        return [
            {
                name: np.asarray(out_arrs[i]).reshape(NCORES, *out_avals[i].shape)[c]
                for i, name in enumerate(out_names)
            }
            for c in range(NCORES)
        ]

    _cache["runner"] = run
    return run


def kernel(x, k0, k1, k2, b0, b1, b2):
    """Full inputs in, full output out. x: (8192, 64, 32) f32 -> (8192, 384) f32."""
    x = np.asarray(x, dtype=np.float32)
    k0 = np.asarray(k0, dtype=np.float32)
    k1 = np.asarray(k1, dtype=np.float32)
    k2 = np.asarray(k2, dtype=np.float32)
    b0 = np.asarray(b0, dtype=np.float32)
    b1 = np.asarray(b1, dtype=np.float32)
    b2 = np.asarray(b2, dtype=np.float32)

    run = _get_runner()
    in_maps = []
    for c in range(NCORES):
        shard = x[BCORE*c:BCORE*(c+1)]
        in_maps.append(_prep_inputs(shard, k0, k1, k2, b0, b1, b2,
                                    l0sym=_CFG["l0sym"]))
    results = run(in_maps)
    out = np.empty((NCORES * BCORE, 384), np.float32)
    for c in range(NCORES):
        r = results[c]
        out[BCORE*c:BCORE*(c+1), 0:128] = r["p0"].T
        out[BCORE*c:BCORE*(c+1), 128:256] = r["p1"].T
        out[BCORE*c:BCORE*(c+1), 256:384] = r["p2"].T
    return out



# revision 35
# speedup vs baseline: 1.1183x; 1.1183x over previous
"""CIN (Compressed Interaction Network) kernel for Trainium2, 8 NeuronCores.

Computes, per reference:
    x0 = xi = x                                  # (8192, 64, 32) fp32
    for (K, b) in layers:                        # K: (k, g, h)
        fm = relu(einsum('bgd,bhd,kgh->bkd', xi, x0, K) + b)
        pooled_i = fm.sum(-1); xi = fm
    out = concat(pooled, -1)                     # (8192, 384) fp32

Strategy (data-parallel over batch across 8 cores, 1024 rows each):
  Per n-tile (N=512 = 16 batch x 32 depth positions), the bilinear term is a
  single long PE accumulation  fm = Kperm @ Z  over 128-row "chunks" of
  Z[(g,h), n] = xi[g,n] * x0[h,n]  (bf16).  Z chunks are formed mostly on the
  Vector engine as wide tensor_tensor multiplies between the running feature
  map and "B tiles" holding broadcast/rotated copies of x0 rows, produced for
  free by the DMA engines straight from DRAM (0-stride / sliding-window access
  patterns).  A slice of layer-0's Z chunks (whose inputs are pure DMA
  products, available at tile start) is offloaded to the idle GPSIMD/Pool
  engine, interleaved v8/p4 so the PE never waits on the slower producer.
  Layer 0 exploits Z = x0 (x) x0 symmetry: K is folded on the host into 33
  diagonal bands, nearly halving layer-0 work.  ReLU+bias runs on the Scalar
  engine out of PSUM; per-layer emission is split into a main phase (Z +
  matmuls + ReLU + half-swap DMA) and a deferred post phase (accum_out pooled
  sums + output DMA) so the next tile's ReLU — on the fm critical path — is
  never queued behind off-path pooling work on the Scalar engine.  Weights
  stay resident in SBUF (bf16, host-permuted).
"""
import numpy as np
import ml_dtypes

import concourse.mybir as mybir
from concourse import bacc, tile

BF16 = mybir.dt.bfloat16
F32 = mybir.dt.float32
N = 512           # n-tile: 16 b * 32 d
NB = 16           # batch rows per tile
BCORE = 1024      # batch rows per core
NTILES = BCORE // NB
NCORES = 8

_CFG = dict(z_bufs=6, b_bufs=2, f_bufs=2, p_bufs=4, zway=8, b_split=1,
            waves=2, swap_dma=1, l0sym=1, act_pool=2, pool_w=4,
            zp_bufs=2, pool_l0=4, pool_tail=0)


def _v3(ap):
    return ap.rearrange("p (b d) -> p b d", d=32)


def _band_src(xt, js, j0, j1, b0):
    """Sliding-window src AP: out[p, j-j0, b, d] = xt[js + 2*j + p, b0+b, d]."""
    nj = j1 - j0
    base = js + 2 * j0
    s = xt[base:base+64, b0:b0+16, :].unsqueeze(1)
    row = BCORE * 32
    s.ap = mybir.VecI64Pair([[row, 64], [2 * row, nj], [32, 16], [1, 32]])
    return s


def _plan(n, pool_cnt, vway, pway):
    """Segment n chunks into interleaved (engine, width) runs: a 'v' run of
    vway, then a 'p' run (≤ pway) while pool quota remains, repeating.
    First run is always DVE so a layer never opens on the slow engine."""
    segs = []
    c, p_left = 0, pool_cnt
    while c < n:
        w = min(vway, n - c - p_left)
        if w > 0:
            segs.append(("v", w))
            c += w
        if p_left and c < n:
            w = min(pway, p_left, n - c)
            segs.append(("p", w))
            c += w
            p_left -= w
    return segs


def _plan_late(n, pool_cnt, vway, pway):
    """Pool runs placed LATE: DVE covers the head, Pool covers pool_cnt
    chunks just before a final DVE run of vway. Pool chunks are consumed
    mid-layer (max head-start after the fm dependency clears) and the
    stop-accumulation chunk stays DVE-produced."""
    if not pool_cnt:
        return _plan(n, 0, vway, pway)
    head = n - pool_cnt - vway
    assert head >= 0
    segs = []
    c = 0
    while c < head:
        w = min(vway, head - c)
        segs.append(("v", w))
        c += w
    p_left = pool_cnt
    while p_left:
        w = min(pway, p_left)
        segs.append(("p", w))
        p_left -= w
    segs.append(("v", vway))
    return segs


def _build(reps=1, ntiles=NTILES, z_bufs=4, b_bufs=2, zway=4, b_split=2,
           waves=1, f_bufs=2, p_bufs=3, swap_dma=0, l0sym=0, act_pool=0,
           pool_w=0, zp_bufs=3, pool_l0=0, pool_tail=0):
    nc = bacc.Bacc("TRN2", target_bir_lowering=False, debug=False)
    nxt = 64 + 33 if l0sym else 64
    xt = nc.dram_tensor("xt", (nxt, BCORE, 32), BF16, kind="ExternalInput").ap()
    nkf0 = 17 if l0sym else 32
    kf0 = nc.dram_tensor("kf0", (128, nkf0, 128), BF16, kind="ExternalInput").ap()
    kf1 = nc.dram_tensor("kf1", (128, 64, 128), BF16, kind="ExternalInput").ap()
    kf2 = nc.dram_tensor("kf2", (128, 64, 128), BF16, kind="ExternalInput").ap()
    bias = nc.dram_tensor("bias", (128, 3), F32, kind="ExternalInput").ap()
    pout = [nc.dram_tensor(f"p{i}", (128, BCORE), F32, kind="ExternalOutput").ap()
            for i in range(3)]

    with tile.TileContext(nc) as tc:
        from contextlib import ExitStack
        with ExitStack() as ctx:
            kpool = ctx.enter_context(tc.tile_pool(name="konst", bufs=1))
            bpool = ctx.enter_context(tc.tile_pool(name="bb", bufs=b_bufs))
            xpool = ctx.enter_context(tc.tile_pool(name="xx", bufs=2 * waves))
            fpool = ctx.enter_context(tc.tile_pool(name="fm", bufs=f_bufs))
            zpool = ctx.enter_context(tc.tile_pool(name="zz", bufs=z_bufs))
            zppool = (ctx.enter_context(tc.tile_pool(name="zzp", bufs=zp_bufs))
                      if pool_w else None)
            opool = ctx.enter_context(tc.tile_pool(name="oo", bufs=3))
            ppool = ctx.enter_context(tc.tile_pool(name="ps", bufs=p_bufs,
                                                   space="PSUM"))

            kt = [kpool.tile([128, nkf0, 128], BF16, tag="kf0", name="kt0"),
                  kpool.tile([128, 64, 128], BF16, tag="kf1", name="kt1"),
                  kpool.tile([128, 64, 128], BF16, tag="kf2", name="kt2")]
            nc.scalar.dma_start(kt[0][:], kf0[:])
            nc.scalar.dma_start(kt[1][:], kf1[:])
            nc.scalar.dma_start(kt[2][:], kf2[:])
            bt = kpool.tile([128, 3], F32, tag="bias", name="bt")
            nc.scalar.dma_start(bt[:], bias[:])

            def tile_dma(t):
                b0 = NB * t
                x2 = xpool.tile([128, N], BF16, tag="x2", name="x2")
                nc.sync.dma_start(_v3(x2[0:64, :]), xt[0:64, b0:b0+NB, :])
                nc.sync.dma_start(_v3(x2[64:128, :]), xt[0:64, b0:b0+NB, :])
                B0 = None
                if l0sym:
                    # B0[p<64, j] = x0[(p+2j)%64]; B0[p>=64, j] = x0[(p-64+2j+1)%64]
                    B0 = bpool.tile([128, 17, N], BF16, tag="B0", name="B0")
                    for lo, hi, js in ((0, 64, 0), (64, 128, 1)):
                        for j0, j1 in ((0, 9), (9, 17)):
                            nc.sync.dma_start(
                                B0[lo:hi, j0:j1, :].rearrange(
                                    "p j (b d) -> p j b d", d=32),
                                _band_src(xt, js, j0, j1, b0))
                # B[p<64, c] = x0[2c] bcast; B[p>=64, c] = x0[2c+1] bcast
                B = bpool.tile([128, 32, N], BF16, tag="B", name="B")
                cs = 32 // b_split
                for s in range(b_split):
                    c0 = cs * s
                    src_e = xt[2*c0:2*(c0+cs):2, b0:b0+NB, :].unsqueeze(0)
                    src_o = xt[2*c0+1:2*(c0+cs):2, b0:b0+NB, :].unsqueeze(0)
                    nc.sync.dma_start(
                        B[0:64, c0:c0+cs, :].rearrange("p c (b d) -> p c b d", d=32),
                        src_e.to_broadcast((64, cs, NB, 32)))
                    nc.sync.dma_start(
                        B[64:128, c0:c0+cs, :].rearrange("p c (b d) -> p c b d", d=32),
                        src_o.to_broadcast((64, cs, NB, 32)))
                return {"x2": x2, "B": B, "B0": B0, "fms": [], "psums": {}}

            def z_chunks(B, a_tile, kf_tile, kf_off, psum, start, stop, nchunk,
                         plan):
                c = 0
                for eng, w in plan:
                    if eng == "v":
                        z = zpool.tile([128, zway, N], BF16, tag="z", name="z")
                        nc.vector.tensor_tensor(
                            z[:, 0:w, :],
                            a_tile[:].unsqueeze(1).to_broadcast((128, w, N)),
                            B[:, c:c+w, :], mybir.AluOpType.mult)
                    else:
                        z = zppool.tile([128, pool_w, N], BF16, tag="zp",
                                        name="zp")
                        nc.gpsimd.tensor_tensor(
                            z[:, 0:w, :],
                            a_tile[:].unsqueeze(1).to_broadcast((128, w, N)),
                            B[:, c:c+w, :], mybir.AluOpType.mult)
                    for i in range(w):
                        nc.tensor.matmul(
                            psum[:], kf_tile[:, kf_off + c + i, :], z[:, i, :],
                            start=(start and c + i == 0),
                            stop=(stop and c + i == nchunk - 1))
                    c += w
                assert c == nchunk

            def tile_layer_main(t, layer, st):
                psum = ppool.tile([128, N], F32, tag="psum", name="psum")
                if layer == 0:
                    if l0sym:
                        z_chunks(st["B0"], st["x2"], kt[0], 0, psum, True, True,
                                 17, _plan(17, pool_l0, zway, pool_w))
                    else:
                        z_chunks(st["B"], st["x2"], kt[0], 0, psum, True, True,
                                 32, _plan(32, pool_l0, zway, pool_w))
                else:
                    fm_prev, fm_prev_s = st["fms"][-1]
                    z_chunks(st["B"], fm_prev, kt[layer], 0, psum, True, False,
                             32, _plan_late(32, pool_tail, zway, pool_w))
                    z_chunks(st["B"], fm_prev_s, kt[layer], 32, psum, False,
                             True, 32, _plan_late(32, pool_tail, zway, pool_w))
                fm = fpool.tile([128, N], BF16, tag=f"fm{layer}", name="fm")
                nc.scalar.activation(fm[:], psum[:],
                                     mybir.ActivationFunctionType.Relu,
                                     bias=bt[:, layer:layer+1], scale=1.0)
                if layer < 2:
                    fm_s = fpool.tile([128, N], BF16, tag=f"fms{layer}", name="fms")
                    if swap_dma:
                        nc.scalar.dma_start(fm_s[0:64, :], fm[64:128, :])
                        nc.scalar.dma_start(fm_s[64:128, :], fm[0:64, :])
                    else:
                        nc.vector.tensor_copy(fm_s[0:64, :], fm[64:128, :])
                        nc.vector.tensor_copy(fm_s[64:128, :], fm[0:64, :])
                    st["fms"].append((fm, fm_s))
                st["psums"][layer] = (psum, fm)

            def tile_layer_post(t, layer, st):
                b0 = NB * t
                psum, fm = st["psums"][layer]
                po = opool.tile([128, NB], F32, tag="po", name="po")
                if act_pool == 2:
                    scr = opool.tile([128, 32], BF16, tag="scr", name="scr")
                    for bb in range(NB):
                        nc.scalar.activation(scr[:],
                                             psum[:, 32*bb:32*(bb+1)],
                                             mybir.ActivationFunctionType.Relu,
                                             bias=bt[:, layer:layer+1],
                                             scale=1.0,
                                             accum_out=po[:, bb:bb+1])
                else:
                    nc.vector.tensor_reduce(po[:], _v3(fm[:]),
                                            axis=mybir.AxisListType.X,
                                            op=mybir.AluOpType.add)
                nc.scalar.dma_start(pout[layer][:, b0:b0+NB], po[:])

            def emit_all():
                for base in range(0, ntiles, waves):
                    ts = [base + w for w in range(waves) if base + w < ntiles]
                    states = [tile_dma(t) for t in ts]
                    for layer in range(3):
                        for t, st in zip(ts, states):
                            tile_layer_main(t, layer, st)
                        for t, st in zip(ts, states):
                            tile_layer_post(t, layer, st)

            if reps > 1:
                with tc.For_i(0, reps, 1):
                    emit_all()
            else:
                emit_all()

    nc.compile()
    return nc


def _prep_inputs(x_shard, k0, k1, k2, b0, b1, b2, l0sym):
    xt = np.ascontiguousarray(x_shard.transpose(1, 0, 2)).astype(ml_dtypes.bfloat16)
    if l0sym:
        xt = np.concatenate([xt, xt[0:33]], axis=0)

    def perm0(K):
        KT = K.astype(np.float32)
        out = np.empty((128, 32, 128), np.float32)
        for c in range(32):
            out[0:64, c, :] = KT[:, :, 2*c].T
            out[64:128, c, :] = KT[:, :, 2*c+1].T
        return out.astype(ml_dtypes.bfloat16)

    def perm0_sym(K):
        # 33 diagonal bands (d=0..32) + zero pad band; chunk j = bands (2j, 2j+1)
        Kf = K.astype(np.float32)
        g = np.arange(64)
        bands = np.zeros((34, 128, 64), np.float32)
        for d in range(33):
            h = (g + d) % 64
            if d == 0:
                bands[d] = Kf[:, g, g]
            elif d == 32:
                bands[d] = Kf[:, g, h]
            else:
                bands[d] = Kf[:, g, h] + Kf[:, h, g]
        out = np.zeros((128, 17, 128), np.float32)
        for j in range(17):
            out[0:64, j, :] = bands[2*j].T
            if 2*j + 1 < 34:
                out[64:128, j, :] = bands[2*j+1].T
        return out.astype(ml_dtypes.bfloat16)

    def perm12(K):
        # normal chunk c: [(g 0:64, h=2c); (g 64:128, h=2c+1)]
        # swap   chunk c: [(g 64:128, h=2c); (g 0:64, h=2c+1)]  (A = fm halves swapped)
        KT = K.astype(np.float32)
        out = np.empty((128, 64, 128), np.float32)
        for c in range(32):
            e = KT[:, :, 2*c].T
            o = KT[:, :, 2*c+1].T
            out[0:64, c, :] = e[0:64]
            out[64:128, c, :] = o[64:128]
            out[0:64, 32+c, :] = e[64:128]
            out[64:128, 32+c, :] = o[0:64]
        return out.astype(ml_dtypes.bfloat16)

    bias = np.stack([np.broadcast_to(b, (128,)) for b in (b0, b1, b2)],
                    axis=1).astype(np.float32)
    return {"xt": xt, "kf0": (perm0_sym(k0) if l0sym else perm0(k0)),
            "kf1": perm12(k1), "kf2": perm12(k2),
            "bias": np.ascontiguousarray(bias)}


_cache = {}


def _get_runner():
    """Build + compile the Bass module and a reusable jitted SPMD runner."""
    if "runner" in _cache:
        return _cache["runner"]
    import jax
    from jax.sharding import Mesh, PartitionSpec
    from jax.experimental.shard_map import shard_map
    from concourse import bass2jax
    from concourse.bass2jax import _bass_exec_p, partition_id_tensor

    nc = _build(reps=1, ntiles=NTILES, **_CFG)
    bass2jax.install_neuronx_cc_hook()

    partition_name = nc.partition_id_tensor.name if nc.partition_id_tensor else None
    in_names, out_names, out_avals, zero_outs = [], [], [], []
    for alloc in nc.m.functions[0].allocations:
        if not isinstance(alloc, mybir.MemoryLocationSet):
            continue
        name = alloc.memorylocations[0].name
        if alloc.kind == "ExternalInput":
            if name != partition_name:
                in_names.append(name)
        elif alloc.kind == "ExternalOutput":
            out_names.append(name)
            shape = tuple(alloc.tensor_shape)
            dtype = mybir.dt.np(alloc.dtype)
            out_avals.append(jax.core.ShapedArray(shape, dtype))
            zero_outs.append(np.zeros(shape, dtype))
    n_params = len(in_names)
    n_outs = len(out_avals)
    in_names_all = in_names + out_names
    if partition_name is not None:
        in_names_all = in_names_all + [partition_name]
    donate = tuple(range(n_params, n_params + n_outs))

    def _body(*args):
        operands = list(args)
        if partition_name is not None:
            operands.append(partition_id_tensor())
        outs = _bass_exec_p.bind(
            *operands,
            out_avals=tuple(out_avals),
            in_names=tuple(in_names_all),
            out_names=tuple(out_names),
            lowering_input_output_aliases=(),
            sim_require_finite=True,
            sim_require_nnan=True,
            nc=nc,
        )
        return tuple(outs)

    devices = jax.devices()[:NCORES]
    assert len(devices) == NCORES, f"need {NCORES} devices, have {len(devices)}"
    mesh = Mesh(np.asarray(devices), ("core",))
    in_specs = (PartitionSpec("core"),) * (n_params + n_outs)
    out_specs = (PartitionSpec("core"),) * len(out_names)
    sharded = jax.jit(
        shard_map(_body, mesh=mesh, in_specs=in_specs, out_specs=out_specs,
                  check_rep=False),
        donate_argnums=donate, keep_unused=True)

    def run(in_maps):
        per_core = [[np.asarray(m[name]) for name in in_names] for m in in_maps]
        concat_in = [
            np.concatenate([per_core[c][i] for c in range(NCORES)], axis=0)
            for i in range(n_params)
        ]
        concat_zeros = [
            np.zeros((NCORES * z.shape[0], *z.shape[1:]), z.dtype)
            for z in zero_outs
        ]
        out_arrs = sharded(*concat_in, *concat_zeros)
        jax.block_until_ready(out_arrs)
        return [
            {
                name: np.asarray(out_arrs[i]).reshape(NCORES, *out_avals[i].shape)[c]
                for i, name in enumerate(out_names)
            }
            for c in range(NCORES)
        ]

    _cache["runner"] = run
    return run


def kernel(x, k0, k1, k2, b0, b1, b2):
    """Full inputs in, full output out. x: (8192, 64, 32) f32 -> (8192, 384) f32."""
    x = np.asarray(x, dtype=np.float32)
    k0 = np.asarray(k0, dtype=np.float32)
    k1 = np.asarray(k1, dtype=np.float32)
    k2 = np.asarray(k2, dtype=np.float32)
    b0 = np.asarray(b0, dtype=np.float32)
    b1 = np.asarray(b1, dtype=np.float32)
    b2 = np.asarray(b2, dtype=np.float32)

    run = _get_runner()
    in_maps = []
    for c in range(NCORES):
        shard = x[BCORE*c:BCORE*(c+1)]
        in_maps.append(_prep_inputs(shard, k0, k1, k2, b0, b1, b2,
                                    l0sym=_CFG["l0sym"]))
    results = run(in_maps)
    out = np.empty((NCORES * BCORE, 384), np.float32)
    for c in range(NCORES):
        r = results[c]
        out[BCORE*c:BCORE*(c+1), 0:128] = r["p0"].T
        out[BCORE*c:BCORE*(c+1), 128:256] = r["p1"].T
        out[BCORE*c:BCORE*(c+1), 256:384] = r["p2"].T
    return out



# revision 36
# speedup vs baseline: 1.1390x; 1.0185x over previous
"""CIN (Compressed Interaction Network) kernel for Trainium2, 8 NeuronCores.

Computes, per reference:
    x0 = xi = x                                  # (8192, 64, 32) fp32
    for (K, b) in layers:                        # K: (k, g, h)
        fm = relu(einsum('bgd,bhd,kgh->bkd', xi, x0, K) + b)
        pooled_i = fm.sum(-1); xi = fm
    out = concat(pooled, -1)                     # (8192, 384) fp32

Strategy (data-parallel over batch across 8 cores, 1024 rows each):
  Per n-tile (N=512 = 16 batch x 32 depth positions), the bilinear term is a
  single long PE accumulation  fm = Kperm @ Z  over 128-row "chunks" of
  Z[(g,h), n] = xi[g,n] * x0[h,n]  (bf16).  Z chunks are formed mostly on the
  Vector engine as wide tensor_tensor multiplies between the running feature
  map and "B tiles" holding broadcast/rotated copies of x0 rows, produced for
  free by the DMA engines straight from DRAM (0-stride / sliding-window access
  patterns).  A slice of layer-0's Z chunks (whose inputs are pure DMA
  products, available at tile start) is offloaded to the idle GPSIMD/Pool
  engine, interleaved v8/p4 so the PE never waits on the slower producer.
  Layer 0 exploits Z = x0 (x) x0 symmetry: K is folded on the host into 33
  diagonal bands, nearly halving layer-0 work.  ReLU+bias runs on the Scalar
  engine out of PSUM; per-layer emission is split into a main phase (Z +
  matmuls + ReLU + half-swap DMA) and a deferred post phase (accum_out pooled
  sums + output DMA) so the next tile's ReLU — on the fm critical path — is
  never queued behind off-path pooling work on the Scalar engine.  Weights
  stay resident in SBUF (bf16, host-permuted).
"""
import numpy as np
import ml_dtypes

import concourse.mybir as mybir
from concourse import bacc, tile

BF16 = mybir.dt.bfloat16
F32 = mybir.dt.float32
N = 512           # n-tile: 16 b * 32 d
NB = 16           # batch rows per tile
BCORE = 1024      # batch rows per core
NTILES = BCORE // NB
NCORES = 8

_CFG = dict(z_bufs=6, b_bufs=2, f_bufs=2, p_bufs=4, zway=8, b_split=1,
            waves=2, swap_dma=1, l0sym=1, act_pool=2, pool_w=4,
            zp_bufs=3, pool_l0=4, pool_tail=0)


def _v3(ap):
    return ap.rearrange("p (b d) -> p b d", d=32)


def _band_src(xt, js, j0, j1, b0):
    """Sliding-window src AP: out[p, j-j0, b, d] = xt[js + 2*j + p, b0+b, d]."""
    nj = j1 - j0
    base = js + 2 * j0
    s = xt[base:base+64, b0:b0+16, :].unsqueeze(1)
    row = BCORE * 32
    s.ap = mybir.VecI64Pair([[row, 64], [2 * row, nj], [32, 16], [1, 32]])
    return s


def _plan(n, pool_cnt, vway, pway):
    """Segment n chunks into interleaved (engine, width) runs: a 'v' run of
    vway, then a 'p' run (≤ pway) while pool quota remains, repeating.
    First run is always DVE so a layer never opens on the slow engine."""
    segs = []
    c, p_left = 0, pool_cnt
    while c < n:
        w = min(vway, n - c - p_left)
        if w > 0:
            segs.append(("v", w))
            c += w
        if p_left and c < n:
            w = min(pway, p_left, n - c)
            segs.append(("p", w))
            c += w
            p_left -= w
    return segs


def _plan_late(n, pool_cnt, vway, pway):
    """Pool runs placed LATE: DVE covers the head, Pool covers pool_cnt
    chunks just before a final DVE run of vway. Pool chunks are consumed
    mid-layer (max head-start after the fm dependency clears) and the
    stop-accumulation chunk stays DVE-produced."""
    if not pool_cnt:
        return _plan(n, 0, vway, pway)
    head = n - pool_cnt - vway
    assert head >= 0
    segs = []
    c = 0
    while c < head:
        w = min(vway, head - c)
        segs.append(("v", w))
        c += w
    p_left = pool_cnt
    while p_left:
        w = min(pway, p_left)
        segs.append(("p", w))
        p_left -= w
    segs.append(("v", vway))
    return segs


def _build(reps=1, ntiles=NTILES, z_bufs=4, b_bufs=2, zway=4, b_split=2,
           waves=1, f_bufs=2, p_bufs=3, swap_dma=0, l0sym=0, act_pool=0,
           pool_w=0, zp_bufs=3, pool_l0=0, pool_tail=0):
    nc = bacc.Bacc("TRN2", target_bir_lowering=False, debug=False)
    nxt = 64 + 33 if l0sym else 64
    xt = nc.dram_tensor("xt", (nxt, BCORE, 32), BF16, kind="ExternalInput").ap()
    nkf0 = 17 if l0sym else 32
    kf0 = nc.dram_tensor("kf0", (128, nkf0, 128), BF16, kind="ExternalInput").ap()
    kf1 = nc.dram_tensor("kf1", (128, 64, 128), BF16, kind="ExternalInput").ap()
    kf2 = nc.dram_tensor("kf2", (128, 64, 128), BF16, kind="ExternalInput").ap()
    bias = nc.dram_tensor("bias", (128, 3), F32, kind="ExternalInput").ap()
    pout = [nc.dram_tensor(f"p{i}", (128, BCORE), F32, kind="ExternalOutput").ap()
            for i in range(3)]

    with tile.TileContext(nc) as tc:
        from contextlib import ExitStack
        with ExitStack() as ctx:
            kpool = ctx.enter_context(tc.tile_pool(name="konst", bufs=1))
            bpool = ctx.enter_context(tc.tile_pool(name="bb", bufs=b_bufs))
            xpool = ctx.enter_context(tc.tile_pool(name="xx", bufs=waves + 1))
            fpool = ctx.enter_context(tc.tile_pool(name="fm", bufs=f_bufs))
            zpool = ctx.enter_context(tc.tile_pool(name="zz", bufs=z_bufs))
            zppool = (ctx.enter_context(tc.tile_pool(name="zzp", bufs=zp_bufs))
                      if pool_w else None)
            opool = ctx.enter_context(tc.tile_pool(name="oo", bufs=3))
            ppool = ctx.enter_context(tc.tile_pool(name="ps", bufs=p_bufs,
                                                   space="PSUM"))

            kt = [kpool.tile([128, nkf0, 128], BF16, tag="kf0", name="kt0"),
                  kpool.tile([128, 64, 128], BF16, tag="kf1", name="kt1"),
                  kpool.tile([128, 64, 128], BF16, tag="kf2", name="kt2")]
            nc.scalar.dma_start(kt[0][:], kf0[:])
            nc.scalar.dma_start(kt[1][:], kf1[:])
            nc.scalar.dma_start(kt[2][:], kf2[:])
            bt = kpool.tile([128, 3], F32, tag="bias", name="bt")
            nc.scalar.dma_start(bt[:], bias[:])

            def tile_dma(t):
                b0 = NB * t
                x2 = xpool.tile([128, N], BF16, tag="x2", name="x2")
                nc.sync.dma_start(_v3(x2[0:64, :]), xt[0:64, b0:b0+NB, :])
                nc.sync.dma_start(_v3(x2[64:128, :]), xt[0:64, b0:b0+NB, :])
                B0 = None
                if l0sym:
                    # B0[p<64, j] = x0[(p+2j)%64]; B0[p>=64, j] = x0[(p-64+2j+1)%64]
                    B0 = bpool.tile([128, 17, N], BF16, tag="B0", name="B0")
                    for lo, hi, js in ((0, 64, 0), (64, 128, 1)):
                        for j0, j1 in ((0, 9), (9, 17)):
                            nc.sync.dma_start(
                                B0[lo:hi, j0:j1, :].rearrange(
                                    "p j (b d) -> p j b d", d=32),
                                _band_src(xt, js, j0, j1, b0))
                # B[p<64, c] = x0[2c] bcast; B[p>=64, c] = x0[2c+1] bcast
                B = bpool.tile([128, 32, N], BF16, tag="B", name="B")
                cs = 32 // b_split
                for s in range(b_split):
                    c0 = cs * s
                    src_e = xt[2*c0:2*(c0+cs):2, b0:b0+NB, :].unsqueeze(0)
                    src_o = xt[2*c0+1:2*(c0+cs):2, b0:b0+NB, :].unsqueeze(0)
                    nc.sync.dma_start(
                        B[0:64, c0:c0+cs, :].rearrange("p c (b d) -> p c b d", d=32),
                        src_e.to_broadcast((64, cs, NB, 32)))
                    nc.sync.dma_start(
                        B[64:128, c0:c0+cs, :].rearrange("p c (b d) -> p c b d", d=32),
                        src_o.to_broadcast((64, cs, NB, 32)))
                return {"x2": x2, "B": B, "B0": B0, "fms": [], "psums": {}}

            def z_chunks(B, a_tile, kf_tile, kf_off, psum, start, stop, nchunk,
                         plan):
                c = 0
                for eng, w in plan:
                    if eng == "v":
                        z = zpool.tile([128, zway, N], BF16, tag="z", name="z")
                        nc.vector.tensor_tensor(
                            z[:, 0:w, :],
                            a_tile[:].unsqueeze(1).to_broadcast((128, w, N)),
                            B[:, c:c+w, :], mybir.AluOpType.mult)
                    else:
                        z = zppool.tile([128, pool_w, N], BF16, tag="zp",
                                        name="zp")
                        nc.gpsimd.tensor_tensor(
                            z[:, 0:w, :],
                            a_tile[:].unsqueeze(1).to_broadcast((128, w, N)),
                            B[:, c:c+w, :], mybir.AluOpType.mult)
                    for i in range(w):
                        nc.tensor.matmul(
                            psum[:], kf_tile[:, kf_off + c + i, :], z[:, i, :],
                            start=(start and c + i == 0),
                            stop=(stop and c + i == nchunk - 1))
                    c += w
                assert c == nchunk

            def tile_layer_main(t, layer, st):
                psum = ppool.tile([128, N], F32, tag="psum", name="psum")
                if layer == 0:
                    if l0sym:
                        z_chunks(st["B0"], st["x2"], kt[0], 0, psum, True, True,
                                 17, _plan(17, pool_l0, zway, pool_w))
                    else:
                        z_chunks(st["B"], st["x2"], kt[0], 0, psum, True, True,
                                 32, _plan(32, pool_l0, zway, pool_w))
                else:
                    fm_prev, fm_prev_s = st["fms"][-1]
                    z_chunks(st["B"], fm_prev, kt[layer], 0, psum, True, False,
                             32, _plan_late(32, pool_tail, zway, pool_w))
                    z_chunks(st["B"], fm_prev_s, kt[layer], 32, psum, False,
                             True, 32, _plan_late(32, pool_tail, zway, pool_w))
                fm = fpool.tile([128, N], BF16, tag=f"fm{layer}", name="fm")
                nc.scalar.activation(fm[:], psum[:],
                                     mybir.ActivationFunctionType.Relu,
                                     bias=bt[:, layer:layer+1], scale=1.0)
                if layer < 2:
                    fm_s = fpool.tile([128, N], BF16, tag=f"fms{layer}", name="fms")
                    if swap_dma:
                        nc.scalar.dma_start(fm_s[0:64, :], fm[64:128, :])
                        nc.scalar.dma_start(fm_s[64:128, :], fm[0:64, :])
                    else:
                        nc.vector.tensor_copy(fm_s[0:64, :], fm[64:128, :])
                        nc.vector.tensor_copy(fm_s[64:128, :], fm[0:64, :])
                    st["fms"].append((fm, fm_s))
                st["psums"][layer] = (psum, fm)

            def tile_layer_post(t, layer, st):
                b0 = NB * t
                psum, fm = st["psums"][layer]
                po = opool.tile([128, NB], F32, tag="po", name="po")
                if act_pool == 2:
                    scr = opool.tile([128, 32], BF16, tag="scr", name="scr")
                    for bb in range(NB):
                        nc.scalar.activation(scr[:],
                                             psum[:, 32*bb:32*(bb+1)],
                                             mybir.ActivationFunctionType.Relu,
                                             bias=bt[:, layer:layer+1],
                                             scale=1.0,
                                             accum_out=po[:, bb:bb+1])
                else:
                    nc.vector.tensor_reduce(po[:], _v3(fm[:]),
                                            axis=mybir.AxisListType.X,
                                            op=mybir.AluOpType.add)
                nc.scalar.dma_start(pout[layer][:, b0:b0+NB], po[:])

            def emit_all():
                for base in range(0, ntiles, waves):
                    ts = [base + w for w in range(waves) if base + w < ntiles]
                    states = [tile_dma(t) for t in ts]
                    for layer in range(3):
                        for t, st in zip(ts, states):
                            tile_layer_main(t, layer, st)
                        for t, st in zip(ts, states):
                            tile_layer_post(t, layer, st)

            if reps > 1:
                with tc.For_i(0, reps, 1):
                    emit_all()
            else:
                emit_all()

    nc.compile()
    return nc


def _prep_inputs(x_shard, k0, k1, k2, b0, b1, b2, l0sym):
    xt = np.ascontiguousarray(x_shard.transpose(1, 0, 2)).astype(ml_dtypes.bfloat16)
    if l0sym:
        xt = np.concatenate([xt, xt[0:33]], axis=0)

    def perm0(K):
        KT = K.astype(np.float32)
        out = np.empty((128, 32, 128), np.float32)
        for c in range(32):
            out[0:64, c, :] = KT[:, :, 2*c].T
            out[64:128, c, :] = KT[:, :, 2*c+1].T
        return out.astype(ml_dtypes.bfloat16)

    def perm0_sym(K):
        # 33 diagonal bands (d=0..32) + zero pad band; chunk j = bands (2j, 2j+1)
        Kf = K.astype(np.float32)
        g = np.arange(64)
        bands = np.zeros((34, 128, 64), np.float32)
        for d in range(33):
            h = (g + d) % 64
            if d == 0:
                bands[d] = Kf[:, g, g]
            elif d == 32:
                bands[d] = Kf[:, g, h]
            else:
                bands[d] = Kf[:, g, h] + Kf[:, h, g]
        out = np.zeros((128, 17, 128), np.float32)
        for j in range(17):
            out[0:64, j, :] = bands[2*j].T
            if 2*j + 1 < 34:
                out[64:128, j, :] = bands[2*j+1].T
        return out.astype(ml_dtypes.bfloat16)

    def perm12(K):
        # normal chunk c: [(g 0:64, h=2c); (g 64:128, h=2c+1)]
        # swap   chunk c: [(g 64:128, h=2c); (g 0:64, h=2c+1)]  (A = fm halves swapped)
        KT = K.astype(np.float32)
        out = np.empty((128, 64, 128), np.float32)
        for c in range(32):
            e = KT[:, :, 2*c].T
            o = KT[:, :, 2*c+1].T
            out[0:64, c, :] = e[0:64]
            out[64:128, c, :] = o[64:128]
            out[0:64, 32+c, :] = e[64:128]
            out[64:128, 32+c, :] = o[0:64]
        return out.astype(ml_dtypes.bfloat16)

    bias = np.stack([np.broadcast_to(b, (128,)) for b in (b0, b1, b2)],
                    axis=1).astype(np.float32)
    return {"xt": xt, "kf0": (perm0_sym(k0) if l0sym else perm0(k0)),
            "kf1": perm12(k1), "kf2": perm12(k2),
            "bias": np.ascontiguousarray(bias)}


_cache = {}


def _get_runner():
    """Build + compile the Bass module and a reusable jitted SPMD runner."""
    if "runner" in _cache:
        return _cache["runner"]
    import jax
    from jax.sharding import Mesh, PartitionSpec
    from jax.experimental.shard_map import shard_map
    from concourse import bass2jax
    from concourse.bass2jax import _bass_exec_p, partition_id_tensor

    nc = _build(reps=1, ntiles=NTILES, **_CFG)
    bass2jax.install_neuronx_cc_hook()

    partition_name = nc.partition_id_tensor.name if nc.partition_id_tensor else None
    in_names, out_names, out_avals, zero_outs = [], [], [], []
    for alloc in nc.m.functions[0].allocations:
        if not isinstance(alloc, mybir.MemoryLocationSet):
            continue
        name = alloc.memorylocations[0].name
        if alloc.kind == "ExternalInput":
            if name != partition_name:
                in_names.append(name)
        elif alloc.kind == "ExternalOutput":
            out_names.append(name)
            shape = tuple(alloc.tensor_shape)
            dtype = mybir.dt.np(alloc.dtype)
            out_avals.append(jax.core.ShapedArray(shape, dtype))
            zero_outs.append(np.zeros(shape, dtype))
    n_params = len(in_names)
    n_outs = len(out_avals)
    in_names_all = in_names + out_names
    if partition_name is not None:
        in_names_all = in_names_all + [partition_name]
    donate = tuple(range(n_params, n_params + n_outs))

    def _body(*args):
        operands = list(args)
        if partition_name is not None:
            operands.append(partition_id_tensor())
        outs = _bass_exec_p.bind(
            *operands,
            out_avals=tuple(out_avals),
            in_names=tuple(in_names_all),
            out_names=tuple(out_names),
            lowering_input_output_aliases=(),
            sim_require_finite=True,
            sim_require_nnan=True,
            nc=nc,
        )
        return tuple(outs)

    devices = jax.devices()[:NCORES]
    assert len(devices) == NCORES, f"need {NCORES} devices, have {len(devices)}"
    mesh = Mesh(np.asarray(devices), ("core",))
    in_specs = (PartitionSpec("core"),) * (n_params + n_outs)
    out_specs = (PartitionSpec("core"),) * len(out_names)
    sharded = jax.jit(
        shard_map(_body, mesh=mesh, in_specs=in_specs, out_specs=out_specs,
                  check_rep=False),
        donate_argnums=donate, keep_unused=True)

    def run(in_maps):
        per_core = [[np.asarray(m[name]) for name in in_names] for m in in_maps]
        concat_in = [
            np.concatenate([per_core[c][i] for c in range(NCORES)], axis=0)
            for i in range(n_params)
        ]
        concat_zeros = [
            np.zeros((NCORES * z.shape[0], *z.shape[1:]), z.dtype)
            for z in zero_outs
        ]
        out_arrs = sharded(*concat_in, *concat_zeros)
        jax.block_until_ready(out_arrs)
        return [
            {
                name: np.asarray(out_arrs[i]).reshape(NCORES, *out_avals[i].shape)[c]
                for i, name in enumerate(out_names)
            }
            for c in range(NCORES)
        ]

    _cache["runner"] = run
    return run


def kernel(x, k0, k1, k2, b0, b1, b2):
    """Full inputs in, full output out. x: (8192, 64, 32) f32 -> (8192, 384) f32."""
    x = np.asarray(x, dtype=np.float32)
    k0 = np.asarray(k0, dtype=np.float32)
    k1 = np.asarray(k1, dtype=np.float32)
    k2 = np.asarray(k2, dtype=np.float32)
    b0 = np.asarray(b0, dtype=np.float32)
    b1 = np.asarray(b1, dtype=np.float32)
    b2 = np.asarray(b2, dtype=np.float32)

    run = _get_runner()
    in_maps = []
    for c in range(NCORES):
        shard = x[BCORE*c:BCORE*(c+1)]
        in_maps.append(_prep_inputs(shard, k0, k1, k2, b0, b1, b2,
                                    l0sym=_CFG["l0sym"]))
    results = run(in_maps)
    out = np.empty((NCORES * BCORE, 384), np.float32)
    for c in range(NCORES):
        r = results[c]
        out[BCORE*c:BCORE*(c+1), 0:128] = r["p0"].T
        out[BCORE*c:BCORE*(c+1), 128:256] = r["p1"].T
        out[BCORE*c:BCORE*(c+1), 256:384] = r["p2"].T
    return out



# revision 37
# speedup vs baseline: 1.1660x; 1.0237x over previous
"""CIN (Compressed Interaction Network) kernel for Trainium2, 8 NeuronCores.

Computes, per reference:
    x0 = xi = x                                  # (8192, 64, 32) fp32
    for (K, b) in layers:                        # K: (k, g, h)
        fm = relu(einsum('bgd,bhd,kgh->bkd', xi, x0, K) + b)
        pooled_i = fm.sum(-1); xi = fm
    out = concat(pooled, -1)                     # (8192, 384) fp32

Strategy (data-parallel over batch across 8 cores, 1024 rows each):
  Per n-tile (N=512 = 16 batch x 32 depth positions), the bilinear term is a
  single long PE accumulation  fm = Kperm @ Z  over 128-row "chunks" of
  Z[(g,h), n] = xi[g,n] * x0[h,n]  (bf16).  Z chunks are formed mostly on the
  Vector engine as wide tensor_tensor multiplies between the running feature
  map and "B tiles" holding broadcast/rotated copies of x0 rows, produced for
  free by the DMA engines straight from DRAM (0-stride / sliding-window access
  patterns).  A slice of layer-0's Z chunks (whose inputs are pure DMA
  products, available at tile start) is offloaded to the idle GPSIMD/Pool
  engine, interleaved v8/p4 so the PE never waits on the slower producer.
  Layer 0 exploits Z = x0 (x) x0 symmetry: K is folded on the host into 33
  diagonal bands, nearly halving layer-0 work.  ReLU+bias runs on the Scalar
  engine out of PSUM; per-layer emission is split into a main phase (Z +
  matmuls + ReLU + half-swap DMA) and a deferred post phase (accum_out pooled
  sums + output DMA) so the next tile's ReLU — on the fm critical path — is
  never queued behind off-path pooling work on the Scalar engine.  Weights
  stay resident in SBUF (bf16, host-permuted).
"""
import numpy as np
import ml_dtypes

import concourse.mybir as mybir
from concourse import bacc, tile

BF16 = mybir.dt.bfloat16
F32 = mybir.dt.float32
N = 512           # n-tile: 16 b * 32 d
NB = 16           # batch rows per tile
BCORE = 1024      # batch rows per core
NTILES = BCORE // NB
NCORES = 8

_CFG = dict(z_bufs=6, b_bufs=2, f_bufs=2, p_bufs=4, zway=8, b_split=2,
            waves=2, swap_dma=1, l0sym=1, act_pool=2, pool_w=4,
            zp_bufs=3, pool_l0=4, pool_tail=0)


def _v3(ap):
    return ap.rearrange("p (b d) -> p b d", d=32)


def _band_src(xt, js, j0, j1, b0):
    """Sliding-window src AP: out[p, j-j0, b, d] = xt[js + 2*j + p, b0+b, d]."""
    nj = j1 - j0
    base = js + 2 * j0
    s = xt[base:base+64, b0:b0+16, :].unsqueeze(1)
    row = BCORE * 32
    s.ap = mybir.VecI64Pair([[row, 64], [2 * row, nj], [32, 16], [1, 32]])
    return s


def _plan(n, pool_cnt, vway, pway):
    """Segment n chunks into interleaved (engine, width) runs: a 'v' run of
    vway, then a 'p' run (≤ pway) while pool quota remains, repeating.
    First run is always DVE so a layer never opens on the slow engine."""
    segs = []
    c, p_left = 0, pool_cnt
    while c < n:
        w = min(vway, n - c - p_left)
        if w > 0:
            segs.append(("v", w))
            c += w
        if p_left and c < n:
            w = min(pway, p_left, n - c)
            segs.append(("p", w))
            c += w
            p_left -= w
    return segs


def _plan_late(n, pool_cnt, vway, pway):
    """Pool runs placed LATE: DVE covers the head, Pool covers pool_cnt
    chunks just before a final DVE run of vway. Pool chunks are consumed
    mid-layer (max head-start after the fm dependency clears) and the
    stop-accumulation chunk stays DVE-produced."""
    if not pool_cnt:
        return _plan(n, 0, vway, pway)
    head = n - pool_cnt - vway
    assert head >= 0
    segs = []
    c = 0
    while c < head:
        w = min(vway, head - c)
        segs.append(("v", w))
        c += w
    p_left = pool_cnt
    while p_left:
        w = min(pway, p_left)
        segs.append(("p", w))
        p_left -= w
    segs.append(("v", vway))
    return segs


def _build(reps=1, ntiles=NTILES, z_bufs=4, b_bufs=2, zway=4, b_split=2,
           waves=1, f_bufs=2, p_bufs=3, swap_dma=0, l0sym=0, act_pool=0,
           pool_w=0, zp_bufs=3, pool_l0=0, pool_tail=0):
    nc = bacc.Bacc("TRN2", target_bir_lowering=False, debug=False)
    nxt = 64 + 33 if l0sym else 64
    xt = nc.dram_tensor("xt", (nxt, BCORE, 32), BF16, kind="ExternalInput").ap()
    nkf0 = 17 if l0sym else 32
    kf0 = nc.dram_tensor("kf0", (128, nkf0, 128), BF16, kind="ExternalInput").ap()
    kf1 = nc.dram_tensor("kf1", (128, 64, 128), BF16, kind="ExternalInput").ap()
    kf2 = nc.dram_tensor("kf2", (128, 64, 128), BF16, kind="ExternalInput").ap()
    bias = nc.dram_tensor("bias", (128, 3), F32, kind="ExternalInput").ap()
    pout = [nc.dram_tensor(f"p{i}", (128, BCORE), F32, kind="ExternalOutput").ap()
            for i in range(3)]

    with tile.TileContext(nc) as tc:
        from contextlib import ExitStack
        with ExitStack() as ctx:
            kpool = ctx.enter_context(tc.tile_pool(name="konst", bufs=1))
            bpool = ctx.enter_context(tc.tile_pool(name="bb", bufs=b_bufs))
            xpool = ctx.enter_context(tc.tile_pool(name="xx", bufs=waves + 1))
            fpool = ctx.enter_context(tc.tile_pool(name="fm", bufs=f_bufs))
            zpool = ctx.enter_context(tc.tile_pool(name="zz", bufs=z_bufs))
            zppool = (ctx.enter_context(tc.tile_pool(name="zzp", bufs=zp_bufs))
                      if pool_w else None)
            opool = ctx.enter_context(tc.tile_pool(name="oo", bufs=3))
            ppool = ctx.enter_context(tc.tile_pool(name="ps", bufs=p_bufs,
                                                   space="PSUM"))

            kt = [kpool.tile([128, nkf0, 128], BF16, tag="kf0", name="kt0"),
                  kpool.tile([128, 64, 128], BF16, tag="kf1", name="kt1"),
                  kpool.tile([128, 64, 128], BF16, tag="kf2", name="kt2")]
            nc.scalar.dma_start(kt[0][:], kf0[:])
            nc.scalar.dma_start(kt[1][:], kf1[:])
            nc.scalar.dma_start(kt[2][:], kf2[:])
            bt = kpool.tile([128, 3], F32, tag="bias", name="bt")
            nc.scalar.dma_start(bt[:], bias[:])

            def tile_dma(t):
                b0 = NB * t
                x2 = xpool.tile([128, N], BF16, tag="x2", name="x2")
                nc.sync.dma_start(_v3(x2[0:64, :]), xt[0:64, b0:b0+NB, :])
                nc.sync.dma_start(_v3(x2[64:128, :]), xt[0:64, b0:b0+NB, :])
                B0 = None
                if l0sym:
                    # B0[p<64, j] = x0[(p+2j)%64]; B0[p>=64, j] = x0[(p-64+2j+1)%64]
                    B0 = bpool.tile([128, 17, N], BF16, tag="B0", name="B0")
                    for lo, hi, js in ((0, 64, 0), (64, 128, 1)):
                        for j0, j1 in ((0, 9), (9, 17)):
                            nc.sync.dma_start(
                                B0[lo:hi, j0:j1, :].rearrange(
                                    "p j (b d) -> p j b d", d=32),
                                _band_src(xt, js, j0, j1, b0))
                # B[p<64, c] = x0[2c] bcast; B[p>=64, c] = x0[2c+1] bcast
                B = bpool.tile([128, 32, N], BF16, tag="B", name="B")
                cs = 32 // b_split
                for s in range(b_split):
                    c0 = cs * s
                    src_e = xt[2*c0:2*(c0+cs):2, b0:b0+NB, :].unsqueeze(0)
                    src_o = xt[2*c0+1:2*(c0+cs):2, b0:b0+NB, :].unsqueeze(0)
                    nc.sync.dma_start(
                        B[0:64, c0:c0+cs, :].rearrange("p c (b d) -> p c b d", d=32),
                        src_e.to_broadcast((64, cs, NB, 32)))
                    nc.sync.dma_start(
                        B[64:128, c0:c0+cs, :].rearrange("p c (b d) -> p c b d", d=32),
                        src_o.to_broadcast((64, cs, NB, 32)))
                return {"x2": x2, "B": B, "B0": B0, "fms": [], "psums": {}}

            def z_chunks(B, a_tile, kf_tile, kf_off, psum, start, stop, nchunk,
                         plan):
                c = 0
                for eng, w in plan:
                    if eng == "v":
                        z = zpool.tile([128, zway, N], BF16, tag="z", name="z")
                        nc.vector.tensor_tensor(
                            z[:, 0:w, :],
                            a_tile[:].unsqueeze(1).to_broadcast((128, w, N)),
                            B[:, c:c+w, :], mybir.AluOpType.mult)
                    else:
                        z = zppool.tile([128, pool_w, N], BF16, tag="zp",
                                        name="zp")
                        nc.gpsimd.tensor_tensor(
                            z[:, 0:w, :],
                            a_tile[:].unsqueeze(1).to_broadcast((128, w, N)),
                            B[:, c:c+w, :], mybir.AluOpType.mult)
                    for i in range(w):
                        nc.tensor.matmul(
                            psum[:], kf_tile[:, kf_off + c + i, :], z[:, i, :],
                            start=(start and c + i == 0),
                            stop=(stop and c + i == nchunk - 1))
                    c += w
                assert c == nchunk

            def tile_layer_main(t, layer, st):
                psum = ppool.tile([128, N], F32, tag="psum", name="psum")
                if layer == 0:
                    if l0sym:
                        z_chunks(st["B0"], st["x2"], kt[0], 0, psum, True, True,
                                 17, _plan(17, pool_l0, zway, pool_w))
                    else:
                        z_chunks(st["B"], st["x2"], kt[0], 0, psum, True, True,
                                 32, _plan(32, pool_l0, zway, pool_w))
                else:
                    fm_prev, fm_prev_s = st["fms"][-1]
                    z_chunks(st["B"], fm_prev, kt[layer], 0, psum, True, False,
                             32, _plan_late(32, pool_tail, zway, pool_w))
                    z_chunks(st["B"], fm_prev_s, kt[layer], 32, psum, False,
                             True, 32, _plan_late(32, pool_tail, zway, pool_w))
                fm = fpool.tile([128, N], BF16, tag=f"fm{layer}", name="fm")
                nc.scalar.activation(fm[:], psum[:],
                                     mybir.ActivationFunctionType.Relu,
                                     bias=bt[:, layer:layer+1], scale=1.0)
                if layer < 2:
                    fm_s = fpool.tile([128, N], BF16, tag=f"fms{layer}", name="fms")
                    if swap_dma:
                        nc.scalar.dma_start(fm_s[0:64, :], fm[64:128, :])
                        nc.scalar.dma_start(fm_s[64:128, :], fm[0:64, :])
                    else:
                        nc.vector.tensor_copy(fm_s[0:64, :], fm[64:128, :])
                        nc.vector.tensor_copy(fm_s[64:128, :], fm[0:64, :])
                    st["fms"].append((fm, fm_s))
                st["psums"][layer] = (psum, fm)

            def tile_layer_post(t, layer, st):
                b0 = NB * t
                psum, fm = st["psums"][layer]
                po = opool.tile([128, NB], F32, tag="po", name="po")
                if act_pool == 2:
                    scr = opool.tile([128, 32], BF16, tag="scr", name="scr")
                    for bb in range(NB):
                        nc.scalar.activation(scr[:],
                                             psum[:, 32*bb:32*(bb+1)],
                                             mybir.ActivationFunctionType.Relu,
                                             bias=bt[:, layer:layer+1],
                                             scale=1.0,
                                             accum_out=po[:, bb:bb+1])
                else:
                    nc.vector.tensor_reduce(po[:], _v3(fm[:]),
                                            axis=mybir.AxisListType.X,
                                            op=mybir.AluOpType.add)
                nc.scalar.dma_start(pout[layer][:, b0:b0+NB], po[:])

            def emit_all():
                for base in range(0, ntiles, waves):
                    ts = [base + w for w in range(waves) if base + w < ntiles]
                    states = [tile_dma(t) for t in ts]
                    for layer in range(3):
                        for t, st in zip(ts, states):
                            tile_layer_main(t, layer, st)
                        for t, st in zip(ts, states):
                            tile_layer_post(t, layer, st)

            if reps > 1:
                with tc.For_i(0, reps, 1):
                    emit_all()
            else:
                emit_all()

    nc.compile()
    return nc


def _prep_inputs(x_shard, k0, k1, k2, b0, b1, b2, l0sym):
    xt = np.ascontiguousarray(x_shard.transpose(1, 0, 2)).astype(ml_dtypes.bfloat16)
    if l0sym:
        xt = np.concatenate([xt, xt[0:33]], axis=0)

    def perm0(K):
        KT = K.astype(np.float32)
        out = np.empty((128, 32, 128), np.float32)
        for c in range(32):
            out[0:64, c, :] = KT[:, :, 2*c].T
            out[64:128, c, :] = KT[:, :, 2*c+1].T
        return out.astype(ml_dtypes.bfloat16)

    def perm0_sym(K):
        # 33 diagonal bands (d=0..32) + zero pad band; chunk j = bands (2j, 2j+1)
        Kf = K.astype(np.float32)
        g = np.arange(64)
        bands = np.zeros((34, 128, 64), np.float32)
        for d in range(33):
            h = (g + d) % 64
            if d == 0:
                bands[d] = Kf[:, g, g]
            elif d == 32:
                bands[d] = Kf[:, g, h]
            else:
                bands[d] = Kf[:, g, h] + Kf[:, h, g]
        out = np.zeros((128, 17, 128), np.float32)
        for j in range(17):
            out[0:64, j, :] = bands[2*j].T
            if 2*j + 1 < 34:
                out[64:128, j, :] = bands[2*j+1].T
        return out.astype(ml_dtypes.bfloat16)

    def perm12(K):
        # normal chunk c: [(g 0:64, h=2c); (g 64:128, h=2c+1)]
        # swap   chunk c: [(g 64:128, h=2c); (g 0:64, h=2c+1)]  (A = fm halves swapped)
        KT = K.astype(np.float32)
        out = np.empty((128, 64, 128), np.float32)
        for c in range(32):
            e = KT[:, :, 2*c].T
            o = KT[:, :, 2*c+1].T
            out[0:64, c, :] = e[0:64]
            out[64:128, c, :] = o[64:128]
            out[0:64, 32+c, :] = e[64:128]
            out[64:128, 32+c, :] = o[0:64]
        return out.astype(ml_dtypes.bfloat16)

    bias = np.stack([np.broadcast_to(b, (128,)) for b in (b0, b1, b2)],
                    axis=1).astype(np.float32)
    return {"xt": xt, "kf0": (perm0_sym(k0) if l0sym else perm0(k0)),
            "kf1": perm12(k1), "kf2": perm12(k2),
            "bias": np.ascontiguousarray(bias)}


_cache = {}


def _get_runner():
    """Build + compile the Bass module and a reusable jitted SPMD runner."""
    if "runner" in _cache:
        return _cache["runner"]
    import jax
    from jax.sharding import Mesh, PartitionSpec
    from jax.experimental.shard_map import shard_map
    from concourse import bass2jax
    from concourse.bass2jax import _bass_exec_p, partition_id_tensor

    nc = _build(reps=1, ntiles=NTILES, **_CFG)
    bass2jax.install_neuronx_cc_hook()

    partition_name = nc.partition_id_tensor.name if nc.partition_id_tensor else None
    in_names, out_names, out_avals, zero_outs = [], [], [], []
    for alloc in nc.m.functions[0].allocations:
        if not isinstance(alloc, mybir.MemoryLocationSet):
            continue
        name = alloc.memorylocations[0].name
        if alloc.kind == "ExternalInput":
            if name != partition_name:
                in_names.append(name)
        elif alloc.kind == "ExternalOutput":
            out_names.append(name)
            shape = tuple(alloc.tensor_shape)
            dtype = mybir.dt.np(alloc.dtype)
            out_avals.append(jax.core.ShapedArray(shape, dtype))
            zero_outs.append(np.zeros(shape, dtype))
    n_params = len(in_names)
    n_outs = len(out_avals)
    in_names_all = in_names + out_names
    if partition_name is not None:
        in_names_all = in_names_all + [partition_name]
    donate = tuple(range(n_params, n_params + n_outs))

    def _body(*args):
        operands = list(args)
        if partition_name is not None:
            operands.append(partition_id_tensor())
        outs = _bass_exec_p.bind(
            *operands,
            out_avals=tuple(out_avals),
            in_names=tuple(in_names_all),
            out_names=tuple(out_names),
            lowering_input_output_aliases=(),
            sim_require_finite=True,
            sim_require_nnan=True,
            nc=nc,
        )
        return tuple(outs)

    devices = jax.devices()[:NCORES]
    assert len(devices) == NCORES, f"need {NCORES} devices, have {len(devices)}"
    mesh = Mesh(np.asarray(devices), ("core",))
    in_specs = (PartitionSpec("core"),) * (n_params + n_outs)
    out_specs = (PartitionSpec("core"),) * len(out_names)
    sharded = jax.jit(
        shard_map(_body, mesh=mesh, in_specs=in_specs, out_specs=out_specs,
                  check_rep=False),
        donate_argnums=donate, keep_unused=True)

    def run(in_maps):
        per_core = [[np.asarray(m[name]) for name in in_names] for m in in_maps]
        concat_in = [
            np.concatenate([per_core[c][i] for c in range(NCORES)], axis=0)
            for i in range(n_params)
        ]
        concat_zeros = [
            np.zeros((NCORES * z.shape[0], *z.shape[1:]), z.dtype)
            for z in zero_outs
        ]
        out_arrs = sharded(*concat_in, *concat_zeros)
        jax.block_until_ready(out_arrs)
        return [
            {
                name: np.asarray(out_arrs[i]).reshape(NCORES, *out_avals[i].shape)[c]
                for i, name in enumerate(out_names)
            }
            for c in range(NCORES)
        ]

    _cache["runner"] = run
    return run


def kernel(x, k0, k1, k2, b0, b1, b2):
    """Full inputs in, full output out. x: (8192, 64, 32) f32 -> (8192, 384) f32."""
    x = np.asarray(x, dtype=np.float32)
    k0 = np.asarray(k0, dtype=np.float32)
    k1 = np.asarray(k1, dtype=np.float32)
    k2 = np.asarray(k2, dtype=np.float32)
    b0 = np.asarray(b0, dtype=np.float32)
    b1 = np.asarray(b1, dtype=np.float32)
    b2 = np.asarray(b2, dtype=np.float32)

    run = _get_runner()
    in_maps = []
    for c in range(NCORES):
        shard = x[BCORE*c:BCORE*(c+1)]
        in_maps.append(_prep_inputs(shard, k0, k1, k2, b0, b1, b2,
                                    l0sym=_CFG["l0sym"]))
    results = run(in_maps)
    out = np.empty((NCORES * BCORE, 384), np.float32)
    for c in range(NCORES):
        r = results[c]
        out[BCORE*c:BCORE*(c+1), 0:128] = r["p0"].T
        out[BCORE*c:BCORE*(c+1), 128:256] = r["p1"].T
        out[BCORE*c:BCORE*(c+1), 256:384] = r["p2"].T
    return out

